# revision 1
# baseline (speedup 1.0000x reference)
"""Trainium2 Bass kernel for nn_AttentionLayer (B=8, N=2048, D=512).

Sharding: data-parallel over batch — one batch element per NeuronCore (8 cores),
no collectives.

Per-core pipeline (x_b [2048, 512]), chunk-major over 4 q-chunks of 512 rows so
LN/expand/attention/project of successive chunks overlap:
  1. LayerNorm in natural layout; PE-transpose nx -> nx_T (per-chunk tiles).
  2. Expand GEMM (h = nx @ expand, 2176 cols) split by consumer:
       - q/k/local-linear/local-pregelu computed TRANSPOSED (h_T = expand.T @ nx)
       - v-linear/v-pregelu computed NATURAL (rows on partitions)
     so attention needs no score/v transposes.
  3. Logits transposed [k, q] directly from q_T/k_T; causal sigmoid mask
     precomputed on host (fp16), added on DVE; exp on ACT without
     max-subtraction (logits are O(1)).
  4. attn_T = v.T @ e (unnormalized, 2 PSUM banks x 2 d-passes) + ones-matmul
     row-sum denominator; normalization deferred to after the project GEMM
     (per-partition scalar on the ACT copy).
  5. Project GEMM consumes local_T/attn_T as stationary operands; residual
     added on DVE. Causal structure skips fully-masked k-tiles (40/64 kept).

All matmuls run in float32r (FP22, single-pass full PE rate).
"""

import numpy as np

import concourse.bass as bass
import concourse.mybir as mybir
import concourse.tile as tile
import concourse.bass_utils as bass_utils
from concourse.masks import make_identity
from concourse import bass_isa
from concourse.vector_clock import ScopedClock

F32 = mybir.dt.float32
F32R = mybir.dt.float32r
F16 = mybir.dt.float16
AF = mybir.ActivationFunctionType
ALU = mybir.AluOpType
X_AX = mybir.AxisListType.X

B = 8
N = 2048
D = 512
QK = 64
ED = 1024
OUTE = 2176
LN_EPS = 1e-5
NT = N // 128      # 16 row tiles
KT = D // 128      # 4 contraction tiles (feature dim)
NCH = 4            # q chunks of 512
CH = N // NCH      # 512


# ----------------------------------------------------------------------------
# Workaround for the walrus build in this container: CTRL-class instructions
# (Drain/NoOp) support only ONE sync-wait command. Split multi-wait
# instructions by hoisting extra waits onto preceding same-engine NOPs.
# ----------------------------------------------------------------------------
_SPLIT_LIMIT = 1
_patched = [False]


def _apply_patches():
    if _patched[0]:
        return
    _patched[0] = True

    orig_add = tile.TileContext._add_instruction
    ctr = [0]

    def _split_add(self, inst):
        si = inst.sync_info
        if (si is not None and si.on_wait and len(si.on_wait) > _SPLIT_LIMIT
                and inst.engine != mybir.EngineType.Unassigned):
            waits = list(si.on_wait)
            for w in waits[:-_SPLIT_LIMIT]:
                ctr[0] += 1
                nop = mybir.InstNoOp(name=f"I-waitsplit-{ctr[0]}", ins=[], outs=[])
                nop.engine = inst.engine
                nop.sync_info = mybir.SyncInfo(on_wait=[w], on_update=[])
                orig_add(self, nop)
            si.on_wait = waits[-_SPLIT_LIMIT:]
        orig_add(self, inst)

    tile.TileContext._add_instruction = _split_add

    def _patched_drain_and_barrier(self, tick_clock, wait_clock):
        nc = self.nc
        drain_inst = nc.sync.drain()
        wait_clock.add_sem_waits(
            drain_inst.ins, ScopedClock({None: tick_clock.global_clock})
        )
        si = drain_inst.ins.sync_info
        if si is not None and si.on_wait and len(si.on_wait) > _SPLIT_LIMIT:
            waits = list(si.on_wait)
            si.on_wait = waits[:_SPLIT_LIMIT]
            for w in waits[_SPLIT_LIMIT:]:
                d2 = nc.sync.drain()
                s2 = d2.ins.sync_info
                if s2 is None:
                    d2.ins.sync_info = mybir.SyncInfo(on_wait=[w], on_update=[])
                else:
                    s2.on_wait = [w]
        nc.all_engine_barrier()
        popped = nc._tile_sem_poison_stack.pop()
        assert popped is self._sem_poison
        nc.clear_and_free_semaphores(list(self.sems.allocated().values()))
        nc.all_engine_barrier()

    tile.TileContext._drain_and_barrier = _patched_drain_and_barrier


def _emit(nc, tc):
    x = nc.dram_tensor("x", [N, D], F32, kind="ExternalInput").ap()
    expd = nc.dram_tensor("expand", [D, OUTE], F32, kind="ExternalInput").ap()
    projd = nc.dram_tensor("project", [ED, D], F32, kind="ExternalInput").ap()
    maskd = nc.dram_tensor("maskT", [N, N], F16, kind="ExternalInput").ap()
    y = nc.dram_tensor("y", [N, D], F32, kind="ExternalOutput").ap()

    from contextlib import ExitStack
    with ExitStack() as _ctx:
        def _pool(name, bufs, space="SBUF"):
            return _ctx.enter_context(
                tc.tile_pool(name=name, bufs=bufs, space=space))

        constp = _pool("constp", 1)
        pp = _pool("pp", 1)
        wp = _pool("wp", 1)
        xpp = _pool("xpp", 8)
        nxTp = _pool("nxTp", 8)
        qp = _pool("qp", 2)
        gltp = _pool("gltp", 2)
        ep = _pool("ep", 6)
        lmp = _pool("lmp", 3)
        mkp = _pool("mkp", 4)
        asbp = _pool("asbp", 1)
        misc = _pool("misc", 2)
        stp = _pool("stp", 4)
        wsp = _pool("wsp", 4)
        outp = _pool("outp", 2)
        denp = _pool("denp", 1)
        dramp = _pool("dramp", 1, space="DRAM")
        ps1 = _pool("ps1", 2, space="PSUM")
        psL = _pool("psL", 1, space="PSUM")
        psB = _pool("psB", 1, space="PSUM")
        psD = _pool("psD", 1, space="PSUM")
        psD = _pool("psD", 1, space="PSUM")

        ident = constp.tile([128, 128], F32, tag="ident")
        make_identity(nc, ident)
        ones_f = constp.tile([128, 1], F32, tag="ones_f")
        nc.vector.memset(ones_f, 1.0)
        ones = constp.tile([128, 1], F32R, tag="ones")
        nc.vector.tensor_copy(ones, ones_f)
        epst = constp.tile([128, 1], F32, tag="epst")
        nc.vector.memset(epst, LN_EPS)

        # persistent across chunks
        k_all = [pp.tile([64, CH], F32R, tag=f"k{c}", name=f"k{c}")
                 for c in range(NCH)]
        v_sb = [pp.tile([128, D], F32R, tag=f"v{r}", name=f"v{r}")
                for r in range(NT)]
        den_flat = dramp.tile([1, N], F32, name="den_flat")

        def ln_chunk(c):
            nxT = [nxTp.tile([128, CH], F32R, tag="nxT", name=f"nxT{c}_{kt}")
                   for kt in range(KT)]
            x_tiles = []
            for t in range(4):
                r = 4 * c + t
                xt = xpp.tile([128, D], F32, tag="x", name=f"x_{r}")
                nc.sync.dma_start(xt, x[r * 128:(r + 1) * 128, :])
                x_tiles.append(xt)
                mu = stp.tile([128, 1], F32, tag="mu")
                nc.vector.reduce_sum(out=mu, in_=xt, axis=X_AX)
                sq = misc.tile([128, D], F32, tag="sq", bufs=1)
                ssq = stp.tile([128, 1], F32, tag="ssq")
                nc.scalar.activation(sq, xt, AF.Square, accum_out=ssq)
                nc.vector.tensor_scalar_mul(mu, mu, 1.0 / D)
                musq = stp.tile([128, 1], F32, tag="musq")
                nc.vector.tensor_mul(musq, mu, mu)
                var = stp.tile([128, 1], F32, tag="var")
                nc.vector.tensor_scalar(var, ssq, 1.0 / D, None, op0=ALU.mult)
                nc.vector.tensor_sub(var, var, musq)
                std = stp.tile([128, 1], F32, tag="std")
                nc.scalar.activation(std, var, AF.Sqrt, bias=epst)
                rstd = stp.tile([128, 1], F32, tag="rstd")
                nc.vector.reciprocal(rstd, std)
                nxt = misc.tile([128, D], F32, tag="nx")
                nc.vector.tensor_scalar(nxt, xt, mu, rstd,
                                        op0=ALU.subtract, op1=ALU.mult)
                tp = ps1.tile([128, 512], F32, tag="ps")
                for j in range(KT):
                    nc.tensor.matmul(tp[:, j * 128:(j + 1) * 128],
                                     nxt[:, j * 128:(j + 1) * 128], ident,
                                     is_transpose=True, skip_group_check=True)
                for j in range(KT):
                    nc.scalar.copy(nxT[j][:, t * 128:(t + 1) * 128],
                                   tp[:, j * 128:(j + 1) * 128])
            return nxT, x_tiles

        pend = ln_chunk(0)

        # resident weights: emitted after chunk-0 x DMAs so LN starts first
        expnat = [wp.tile([128, ED], F32R, tag=f"en{kt}", name=f"en{kt}")
                  for kt in range(KT)]
        for kt in range(KT):
            nc.sync.dma_start(expnat[kt][:, 0:512],
                              expd[kt * 128:(kt + 1) * 128, 640:1152].bitcast(F32R))
            nc.sync.dma_start(expnat[kt][:, 512:1024],
                              expd[kt * 128:(kt + 1) * 128, 1664:2176].bitcast(F32R))
        projsb = wp.tile([128, 8 * 512], F32R, tag="projsb")
        for j in range(8):
            nc.sync.dma_start(projsb[:, j * 512:(j + 1) * 512],
                              projd[j * 128:(j + 1) * 128, :].bitcast(F32R))

        for c in range(NCH):
            nxT, x_tiles = pend
            # ---------------- expand T-part for this chunk -----------------
            def t_mm(ps_ap, wts, m):
                for kt in range(KT):
                    nc.tensor.matmul(ps_ap[:m, :], wts[kt], nxT[kt],
                                     start=(kt == 0), stop=(kt == KT - 1))

            def load_w(tag, c0, m):
                wts = []
                for kt in range(KT):
                    w = wsp.tile([128, m], F32R, tag=tag, name=f"{tag}_{c}_{kt}")
                    nc.sync.dma_start(
                        w[:, :],
                        expd[kt * 128:(kt + 1) * 128, c0:c0 + m].bitcast(F32R))
                    wts.append(w)
                return wts

            q_sb = qp.tile([64, CH], F32R, tag="q_sb", name=f"q_{c}")
            wq = load_w("wq", 0, 64)
            ps = ps1.tile([128, 512], F32, tag="ps")
            t_mm(ps, wq, 64)
            nc.scalar.copy(q_sb, ps[:64, :])
            wk = load_w("wq", 64, 64)
            ps = ps1.tile([128, 512], F32, tag="ps")
            t_mm(ps, wk, 64)
            nc.scalar.copy(k_all[c], ps[:64, :])

            glt = []
            for j in range(4):
                wl = load_w("wl", 128 + 128 * j, 128)
                wg = load_w("wg", 1152 + 128 * j, 128)
                psl = ps1.tile([128, 512], F32, tag="ps")
                t_mm(psl, wl, 128)
                psg = ps1.tile([128, 512], F32, tag="ps")
                t_mm(psg, wg, 128)
                gelt = misc.tile([128, 512], F32, tag="gelt")
                nc.scalar.activation(gelt, psg, AF.Gelu)
                g = gltp.tile([128, CH], F32R, tag=f"glt{j}", name=f"glt{j}_{c}")
                nc.vector.tensor_mul(g, psl, gelt)
                glt.append(g)

            # ---------------- expand natural part -> v for this chunk ------
            for t in range(4):
                r = 4 * c + t
                pl = ps1.tile([128, 512], F32, tag="ps")
                for kt in range(KT):
                    nc.tensor.matmul(pl, nxT[kt][:, t * 128:(t + 1) * 128],
                                     expnat[kt][:, 0:512],
                                     start=(kt == 0), stop=(kt == KT - 1))
                pg = ps1.tile([128, 512], F32, tag="ps")
                for kt in range(KT):
                    nc.tensor.matmul(pg, nxT[kt][:, t * 128:(t + 1) * 128],
                                     expnat[kt][:, 512:1024],
                                     start=(kt == 0), stop=(kt == KT - 1))
                vg = misc.tile([128, 512], F32, tag="vg")
                nc.scalar.activation(vg, pg, AF.Gelu)
                nc.vector.tensor_mul(v_sb[r], pl, vg)

            if c + 1 < NCH:
                pend = ln_chunk(c + 1)

            # ---------------- attention for this chunk ---------------------
            nkt = 4 * c + 4
            den_ps = psD.tile([1, 512], F32, tag="den")
            attn_ps = [psB.tile([128, 512], F32, tag=f"a{j}", name=f"a{j}_{c}")
                       for j in range(4)]
            for kt in range(nkt):
                lg = psL.tile([128, 512], F32, tag="lg")
                nc.tensor.matmul(lg,
                                 k_all[kt // 4][:, (kt % 4) * 128:
                                                (kt % 4 + 1) * 128],
                                 q_sb)
                mk = mkp.tile([128, 512], F16, tag="mk")
                nc.sync.dma_start(
                    mk, maskd[kt * 128:(kt + 1) * 128, c * CH:(c + 1) * CH])
                lm = lmp.tile([128, 512], F32, tag="lm")
                nc.vector.tensor_add(lm, lg, mk)
                e = ep.tile([128, 512], F32R, tag="e", name=f"e_{c}_{kt}")
                nc.scalar.activation(e, lm, AF.Exp)
                for j in range(4):
                    nc.tensor.matmul(attn_ps[j],
                                     v_sb[kt][:, j * 128:(j + 1) * 128], e,
                                     start=(kt == 0), stop=(kt == nkt - 1))
                nc.tensor.matmul(den_ps, ones, e,
                                 start=(kt == 0), stop=(kt == nkt - 1))

            # denominator -> reciprocal (one Newton step) -> [128, 4]
            r0t = denp.tile([1, 512], F32, tag="r0")
            nc.vector.reciprocal(r0t, den_ps)
            t1 = denp.tile([1, 512], F32, tag="t1")
            nc.vector.tensor_mul(t1, den_ps, r0t)
            nc.vector.tensor_scalar(t1, t1, -1.0, 2.0,
                                    op0=ALU.mult, op1=ALU.add)
            r1t = denp.tile([1, 512], F32, tag="r1")
            nc.vector.tensor_mul(r1t, r0t, t1)
            nc.sync.dma_start(den_flat[0:1, c * CH:(c + 1) * CH], r1t)
            recip_pt = denp.tile([128, 4], F32, tag="recip", bufs=2)
            nc.sync.dma_start(
                recip_pt,
                den_flat[0, c * CH:(c + 1) * CH].rearrange("(t p) -> p t",
                                                           p=128))

            attn_cur = [None] * 4
            for j in range(4):
                asb = asbp.tile([128, 512], F32R, tag=f"as{j}",
                                name=f"as{j}_{c}")
                nc.scalar.copy(asb, attn_ps[j])
                attn_cur[j] = asb

            # ---------------- project + residual for this chunk ------------
            for t in range(4):
                r = 4 * c + t
                o1 = psB.tile([128, 512], F32, tag="a0")
                for j in range(4):
                    nc.tensor.matmul(o1, glt[j][:, t * 128:(t + 1) * 128],
                                     projsb[:, j * 512:(j + 1) * 512],
                                     start=(j == 0), stop=(j == 3))
                o2 = psB.tile([128, 512], F32, tag="a1")
                for j in range(4):
                    nc.tensor.matmul(o2,
                                     attn_cur[j][:, t * 128:(t + 1) * 128],
                                     projsb[:, (4 + j) * 512:(5 + j) * 512],
                                     start=(j == 0), stop=(j == 3))
                a2 = outp.tile([128, 512], F32, tag="a2")
                nc.scalar.activation(a2, o2, AF.Copy,
                                     scale=recip_pt[:, t:t + 1])
                ob = outp.tile([128, 512], F32, tag="ob")
                nc.vector.tensor_add(ob, o1, a2)
                nc.vector.tensor_add(ob, ob, x_tiles[t])
                nc.sync.dma_start(y[r * 128:(r + 1) * 128, :], ob)


_cached = {}


def _build(loop=None):
    import os

    if loop is None:
        loop = int(os.environ.get("ATTN_LOOP", "0"))
    key = ("nc", loop)
    if key in _cached:
        return _cached[key]
    _apply_patches()
    nc = bass.Bass("TRN2", target_bir_lowering=False, debug=False)
    with tile.TileContext(nc) as tc:
        if loop > 1:
            with tc.For_i(0, loop, 1):
                _emit(nc, tc)
        else:
            _emit(nc, tc)
    _cached[key] = nc
    return nc


def _host_prep(expand, project, position_bias_mult):
    E = np.array(expand, dtype=np.float32).copy()
    E[:, :QK] /= np.sqrt(np.float32(QK))  # fold 1/sqrt(dk) into q columns
    pbm = np.float64(position_bias_mult)
    idx = np.arange(N, dtype=np.float64)
    kk = idx[:, None]
    qq = idx[None, :]
    d = kk - qq
    with np.errstate(over="ignore"):
        m = 1.0 / (1.0 + np.exp(-(d + pbm)))
    maskT = np.where(kk <= qq, m, -10000.0).astype(np.float16)
    P = np.array(project, dtype=np.float32)
    return E, P, maskT


def kernel(x, expand, project, position_bias_mult):
    import os

    nc = _build()
    E, P, maskT = _host_prep(expand, project, position_bias_mult)
    xs = np.ascontiguousarray(np.array(x, dtype=np.float32))
    in_maps = [
        {"x": xs[b], "expand": E, "project": P, "maskT": maskT}
        for b in range(B)
    ]
    trace = bool(int(os.environ.get("ATTN_TRACE", "0")))
    res = bass_utils.run_bass_kernel_spmd(
        nc, in_maps, core_ids=list(range(B)), trace=trace)
    _cached["exec_time_ns"] = res.exec_time_ns
    return np.stack([r["y"] for r in res.results], axis=0)

